# revision 7
# baseline (speedup 1.0000x reference)
"""MoE layer (top-2, E=8, capacity-dropped) on 8 TRN2 NeuronCores.

Strategy (expert-parallel):
  - Router (logits -> softmax -> top-2 -> per-expert capacity selection) runs
    on host via jax CPU, mirroring the reference ops exactly so that top-2
    tie-breaks and capacity cutoffs match the reference bit-for-bit.
    (Router flops are 0.06% of total; the MLPs are the compute.)
  - Token dispatch: per expert e, the first `capacity` routed tokens are
    gathered into a [D, TPAD] transposed activation block (the layout the
    TensorEngine wants for lhsT/rhs streaming).
  - Each of the 8 cores runs one expert's dense MLP:
        out = gelu(x @ w1 + b1) @ w2 + b2        (fp32, ~55 GFLOP/core)
    tiled as: chunk tokens (512) -> layer1 produces H^T [DFF, tc] in SBUF
    (weights streamed), gelu fused on ScalarE with b1 bias, layer2 consumes
    H^T tiles as stationary operands with w2 streamed, bias b2 added on the
    PSUM->SBUF move.
  - Host combine: scatter expert outputs back in expert order (later experts
    overwrite), dropped tokens stay zero.
"""

import numpy as np

B, S, D, DFF, E, TOPK = 8, 2048, 1024, 4096, 8, 2
T = B * S                 # 16384 tokens
CAP = 3277                # ceil(T * 1.6 / 8)
TPAD = 3328               # 26 * 128
NOISE_STD = 0.02
N_CORES = 8
CHUNKS = (512, 512, 512, 512, 512, 512, 256)   # sums to TPAD

_CACHE = {}


def _build_nc(d, dff, tpad, chunks, mm_dt_name="float32", act="Gelu"):
    import concourse.mybir as mybir
    import concourse.tile as tile
    from concourse import bacc

    DT = mybir.dt.float32
    MMDT = getattr(mybir.dt, mm_dt_name)
    GELU = getattr(mybir.ActivationFunctionType, act)

    def mm(ap):
        """View a float32 SBUF operand with the matmul compute dtype."""
        return ap if MMDT == DT else ap.bitcast(MMDT)

    kd = d // 128          # k-tiles in D
    nf = dff // 128        # f-tiles in DFF
    nfg = dff // 512       # f-groups of 4
    nd = d // 512          # output n-halves

    nc = bacc.Bacc("TRN2", target_bir_lowering=False, debug=False,
                   num_devices=N_CORES)
    xT_d = nc.dram_tensor("xT", [d, tpad], DT, kind="ExternalInput").ap()
    w1_d = nc.dram_tensor("w1", [d, dff], DT, kind="ExternalInput").ap()
    b1_d = nc.dram_tensor("b1", [dff], DT, kind="ExternalInput").ap()
    w2_d = nc.dram_tensor("w2", [dff, d], DT, kind="ExternalInput").ap()
    b2_d = nc.dram_tensor("b2", [d], DT, kind="ExternalInput").ap()
    out_d = nc.dram_tensor("out", [tpad, d], DT, kind="ExternalOutput").ap()

    with tile.TileContext(nc) as tc:
        with (
            tc.tile_pool(name="consts", bufs=1) as const_pool,
            tc.tile_pool(name="xt", bufs=2) as xt_pool,
            tc.tile_pool(name="ht", bufs=1) as ht_pool,
            tc.tile_pool(name="w1p", bufs=2) as w1_pool,
            tc.tile_pool(name="w2p", bufs=4) as w2_pool,
            tc.tile_pool(name="outp", bufs=6) as out_pool,
            tc.tile_pool(name="ps1", bufs=2, space="PSUM") as ps1_pool,
            tc.tile_pool(name="ps2", bufs=1, space="PSUM") as ps2_pool,
        ):
            # biases: b1 as [128, nf] (partition = within-f-tile index),
            # b2 broadcast to all 128 partitions.
            b1_sb = const_pool.tile([128, nf], DT, tag="b1")
            nc.sync.dma_start(b1_sb[:], b1_d.rearrange("(f p) -> p f", p=128))
            b2_row = const_pool.tile([1, d], DT, tag="b2row")
            nc.sync.dma_start(b2_row[:], b2_d.rearrange("(a n) -> a n", a=1))
            b2_sb = const_pool.tile([128, d], DT, tag="b2")
            nc.gpsimd.partition_broadcast(b2_sb[:], b2_row[:])

            c0 = 0
            for tc_sz in chunks:
                ntt = tc_sz // 128
                # ---- load x^T chunk: kd tiles of [128, tc_sz]
                xt = xt_pool.tile([128, kd * tc_sz], DT, tag="xt")
                for k in range(kd):
                    nc.sync.dma_start(xt[:, k * tc_sz:(k + 1) * tc_sz],
                                      xT_d[k * 128:(k + 1) * 128, c0:c0 + tc_sz])

                # ---- layer 1: H^T[f-tile, tokens] = gelu(w1_kf.T @ xt_k + b1)
                ht = ht_pool.tile([128, nf * tc_sz], DT, tag="ht")
                for fg in range(nfg):
                    w1t = []
                    for k in range(kd):
                        w = w1_pool.tile([128, 512], DT, tag=f"w1_{k}")
                        nc.sync.dma_start(
                            w[:], w1_d[k * 128:(k + 1) * 128,
                                       fg * 512:(fg + 1) * 512])
                        w1t.append(w)
                    for fi in range(4):
                        f = fg * 4 + fi
                        ps = ps1_pool.tile([128, tc_sz], DT, tag="ps1")
                        for k in range(kd):
                            nc.tensor.matmul(
                                ps[:],
                                lhsT=mm(w1t[k][:, fi * 128:(fi + 1) * 128]),
                                rhs=mm(xt[:, k * tc_sz:(k + 1) * tc_sz]),
                                start=(k == 0), stop=(k == kd - 1))
                        nc.scalar.activation(
                            ht[:, f * tc_sz:(f + 1) * tc_sz], ps[:], GELU,
                            bias=b1_sb[:, f:f + 1])

                # ---- layer 2: out[tokens, :] = H^T.T @ w2 + b2
                ots = [out_pool.tile([128, d], DT, tag="ot", name="ot")
                       for _ in range(ntt)]
                for n in range(nd):
                    pss = [ps2_pool.tile([128, 512], DT, tag=f"ps2_{t}",
                                         name=f"ps2_{t}")
                           for t in range(ntt)]
                    for f in range(nf):
                        w2t = w2_pool.tile([128, 512], DT, tag="w2")
                        nc.sync.dma_start(
                            w2t[:], w2_d[f * 128:(f + 1) * 128,
                                         n * 512:(n + 1) * 512])
                        for t in range(ntt):
                            nc.tensor.matmul(
                                pss[t][:],
                                lhsT=mm(ht[:, f * tc_sz + t * 128:
                                           f * tc_sz + t * 128 + 128]),
                                rhs=mm(w2t[:]),
                                start=(f == 0), stop=(f == nf - 1))
                    for t in range(ntt):
                        nc.vector.tensor_add(
                            ots[t][:, n * 512:(n + 1) * 512], pss[t][:],
                            b2_sb[:, n * 512:(n + 1) * 512])
                for t in range(ntt):
                    nc.sync.dma_start(
                        out_d[c0 + t * 128:c0 + (t + 1) * 128, :], ots[t][:])
                c0 += tc_sz
    nc.compile()
    return nc


MM_DTYPE = "float32"   # or "float32r" (4x faster PE, reduced mul precision)


def _get_nc():
    key = (D, DFF, TPAD, CHUNKS, MM_DTYPE)
    if key not in _CACHE:
        _CACHE[key] = _build_nc(D, DFF, TPAD, CHUNKS, mm_dt_name=MM_DTYPE)
    return _CACHE[key]


def _route(x_flat, noise, router_w, router_b):
    """Mirror of the reference router, on jax CPU (decisions verified to
    match the neuron backend bit-for-bit on this input distribution)."""
    import jax
    import jax.numpy as jnp

    cpu = jax.devices("cpu")[0]
    with jax.default_device(cpu):
        xj = jnp.asarray(x_flat)
        logits = (xj @ jnp.asarray(router_w).T + jnp.asarray(router_b)
                  + jnp.asarray(noise) * NOISE_STD)
        probs = jax.nn.softmax(logits, axis=-1)
        _, topk_idx = jax.lax.top_k(probs, TOPK)
    return np.asarray(topk_idx)


def kernel(x, noise, router_w, router_b, w1, b1, w2, b2):
    from concourse.bass_utils import run_bass_kernel_spmd

    x = np.asarray(x, dtype=np.float32)
    noise = np.asarray(noise, dtype=np.float32)
    router_w = np.asarray(router_w, dtype=np.float32)
    router_b = np.asarray(router_b, dtype=np.float32)
    w1 = np.asarray(w1, dtype=np.float32)
    b1 = np.asarray(b1, dtype=np.float32)
    w2 = np.asarray(w2, dtype=np.float32)
    b2 = np.asarray(b2, dtype=np.float32)

    x_flat = x.reshape(T, D)
    topk_idx = _route(x_flat, noise, router_w, router_b)

    # per-expert token selection (first CAP routed tokens, in token order)
    idx_list = []
    for e in range(E):
        nz = np.flatnonzero((topk_idx == e).any(axis=-1))[:CAP]
        idx_list.append(nz)

    # gather + transpose into [D, TPAD] per expert (dropped/pad slots zero)
    xf_T = np.zeros((D, T + 1), dtype=np.float32)
    xf_T[:, :T] = x_flat.T
    in_maps = []
    for e in range(E):
        xT = np.zeros((D, TPAD), dtype=np.float32)
        nz = idx_list[e]
        xT[:, :len(nz)] = xf_T[:, nz]
        in_maps.append({"xT": xT, "w1": w1[e], "b1": b1[e],
                        "w2": w2[e], "b2": b2[e]})

    nc = _get_nc()
    res = run_bass_kernel_spmd(nc, in_maps, core_ids=list(range(N_CORES)))

    out_flat = np.zeros((T, D), dtype=np.float32)
    for e in range(E):
        nz = idx_list[e]
        out_flat[nz] = res.results[e]["out"][:len(nz)]
    return out_flat.reshape(B, S, D)


# revision 14
# speedup vs baseline: 3.5114x; 3.5114x over previous
"""MoE layer (top-2, E=8, capacity-dropped) on 8 TRN2 NeuronCores.

Strategy (expert-parallel):
  - Router (logits -> softmax -> top-2 -> per-expert capacity selection) runs
    on host via jax CPU, mirroring the reference ops exactly so that top-2
    tie-breaks and capacity cutoffs match the reference bit-for-bit.
    (Router flops are 0.06% of total; the MLPs are the compute.)
  - Token dispatch: per expert e, the first `capacity` routed tokens are
    gathered into a [D, TPAD] transposed activation block (the layout the
    TensorEngine wants for lhsT/rhs streaming).
  - Each of the 8 cores runs one expert's dense MLP:
        out = gelu(x @ w1 + b1) @ w2 + b2        (fp32, ~55 GFLOP/core)
    tiled as: chunk tokens (512) -> layer1 produces H^T [DFF, tc] in SBUF
    (weights streamed), gelu fused on ScalarE with b1 bias, layer2 consumes
    H^T tiles as stationary operands with w2 streamed, bias b2 added on the
    PSUM->SBUF move.
  - Host combine: scatter expert outputs back in expert order (later experts
    overwrite), dropped tokens stay zero.
"""

import numpy as np

B, S, D, DFF, E, TOPK = 8, 2048, 1024, 4096, 8, 2
T = B * S                 # 16384 tokens
CAP = 3277                # ceil(T * 1.6 / 8)
TPAD = 3328               # 26 * 128
NOISE_STD = 0.02
N_CORES = 8
CHUNKS = (512, 512, 512, 512, 512, 512, 256)   # sums to TPAD

_CACHE = {}


def _build_nc(d, dff, tpad, chunks, mm_dt_name="float32", act="Gelu"):
    import concourse.mybir as mybir
    import concourse.tile as tile
    from concourse import bacc

    DT = mybir.dt.float32
    MMDT = getattr(mybir.dt, mm_dt_name)
    GELU = getattr(mybir.ActivationFunctionType, act)

    def mm(ap):
        return ap

    kd = d // 128          # k-tiles in D
    nf = dff // 128        # f-tiles in DFF
    nfg = dff // 512       # f-groups of 4
    nd = d // 512          # output n-halves

    nc = bacc.Bacc("TRN2", target_bir_lowering=False, debug=False,
                   num_devices=N_CORES)
    xT_d = nc.dram_tensor("xT", [d, tpad], MMDT, kind="ExternalInput").ap()
    w1_d = nc.dram_tensor("w1", [d, dff], MMDT, kind="ExternalInput").ap()
    b1_d = nc.dram_tensor("b1", [dff], DT, kind="ExternalInput").ap()
    w2_d = nc.dram_tensor("w2", [dff, d], MMDT, kind="ExternalInput").ap()
    b2_d = nc.dram_tensor("b2", [d], DT, kind="ExternalInput").ap()
    out_d = nc.dram_tensor("out", [tpad, d], DT, kind="ExternalOutput").ap()

    with tile.TileContext(nc) as tc:
        with (
            tc.tile_pool(name="consts", bufs=1) as const_pool,
            tc.tile_pool(name="xt", bufs=2) as xt_pool,
            tc.tile_pool(name="ht", bufs=1) as ht_pool,
            tc.tile_pool(name="w1p", bufs=2) as w1_pool,
            tc.tile_pool(name="w2p", bufs=4) as w2_pool,
            tc.tile_pool(name="outp", bufs=6) as out_pool,
            tc.tile_pool(name="ps1", bufs=2, space="PSUM") as ps1_pool,
            tc.tile_pool(name="ps2", bufs=1, space="PSUM") as ps2_pool,
        ):
            # biases: b1 as [128, nf] (partition = within-f-tile index),
            # b2 broadcast to all 128 partitions.
            b1_sb = const_pool.tile([128, nf], DT, tag="b1")
            nc.sync.dma_start(b1_sb[:], b1_d.rearrange("(f p) -> p f", p=128))
            b2_row = const_pool.tile([1, d], DT, tag="b2row")
            nc.sync.dma_start(b2_row[:], b2_d.rearrange("(a n) -> a n", a=1))
            b2_sb = const_pool.tile([128, d], DT, tag="b2")
            nc.gpsimd.partition_broadcast(b2_sb[:], b2_row[:])

            c0 = 0
            for tc_sz in chunks:
                ntt = tc_sz // 128
                # ---- load x^T chunk: kd tiles of [128, tc_sz]
                xt = xt_pool.tile([128, kd * tc_sz], MMDT, tag="xt")
                for k in range(kd):
                    nc.sync.dma_start(xt[:, k * tc_sz:(k + 1) * tc_sz],
                                      xT_d[k * 128:(k + 1) * 128, c0:c0 + tc_sz])

                # ---- layer 1: H^T[f-tile, tokens] = gelu(w1_kf.T @ xt_k + b1)
                ht = ht_pool.tile([128, nf * tc_sz], MMDT, tag="ht")
                for fg in range(nfg):
                    w1t = []
                    for k in range(kd):
                        w = w1_pool.tile([128, 512], MMDT, tag=f"w1_{k}")
                        nc.sync.dma_start(
                            w[:], w1_d[k * 128:(k + 1) * 128,
                                       fg * 512:(fg + 1) * 512])
                        w1t.append(w)
                    for fi in range(4):
                        f = fg * 4 + fi
                        ps = ps1_pool.tile([128, tc_sz], DT, tag="ps1")
                        for k in range(kd):
                            nc.tensor.matmul(
                                ps[:],
                                lhsT=mm(w1t[k][:, fi * 128:(fi + 1) * 128]),
                                rhs=mm(xt[:, k * tc_sz:(k + 1) * tc_sz]),
                                start=(k == 0), stop=(k == kd - 1))
                        nc.scalar.activation(
                            ht[:, f * tc_sz:(f + 1) * tc_sz], ps[:], GELU,
                            bias=b1_sb[:, f:f + 1])

                # ---- layer 2: out[tokens, :] = H^T.T @ w2 + b2
                ots = [out_pool.tile([128, d], DT, tag="ot", name="ot")
                       for _ in range(ntt)]
                for n in range(nd):
                    pss = [ps2_pool.tile([128, 512], DT, tag=f"ps2_{t}",
                                         name=f"ps2_{t}")
                           for t in range(ntt)]
                    for f in range(nf):
                        w2t = w2_pool.tile([128, 512], MMDT, tag="w2")
                        nc.sync.dma_start(
                            w2t[:], w2_d[f * 128:(f + 1) * 128,
                                         n * 512:(n + 1) * 512])
                        for t in range(ntt):
                            nc.tensor.matmul(
                                pss[t][:],
                                lhsT=mm(ht[:, f * tc_sz + t * 128:
                                           f * tc_sz + t * 128 + 128]),
                                rhs=mm(w2t[:]),
                                start=(f == 0), stop=(f == nf - 1))
                    for t in range(ntt):
                        nc.vector.tensor_add(
                            ots[t][:, n * 512:(n + 1) * 512], pss[t][:],
                            b2_sb[:, n * 512:(n + 1) * 512])
                for t in range(ntt):
                    nc.sync.dma_start(
                        out_d[c0 + t * 128:c0 + (t + 1) * 128, :], ots[t][:])
                c0 += tc_sz
    nc.compile()
    return nc


MM_DTYPE = "float32r"  # 4x faster PE than float32; ~2e-4 rel absmax error


def _get_nc():
    key = (D, DFF, TPAD, CHUNKS, MM_DTYPE)
    if key not in _CACHE:
        _CACHE[key] = _build_nc(D, DFF, TPAD, CHUNKS, mm_dt_name=MM_DTYPE)
    return _CACHE[key]


def _route(x_flat, noise, router_w, router_b):
    """Mirror of the reference router, on jax CPU (decisions verified to
    match the neuron backend bit-for-bit on this input distribution)."""
    import jax
    import jax.numpy as jnp

    cpu = jax.devices("cpu")[0]
    with jax.default_device(cpu):
        xj = jnp.asarray(x_flat)
        logits = (xj @ jnp.asarray(router_w).T + jnp.asarray(router_b)
                  + jnp.asarray(noise) * NOISE_STD)
        probs = jax.nn.softmax(logits, axis=-1)
        _, topk_idx = jax.lax.top_k(probs, TOPK)
    return np.asarray(topk_idx)


def kernel(x, noise, router_w, router_b, w1, b1, w2, b2):
    from concourse.bass_utils import run_bass_kernel_spmd

    x = np.asarray(x, dtype=np.float32)
    noise = np.asarray(noise, dtype=np.float32)
    router_w = np.asarray(router_w, dtype=np.float32)
    router_b = np.asarray(router_b, dtype=np.float32)
    w1 = np.asarray(w1, dtype=np.float32)
    b1 = np.asarray(b1, dtype=np.float32)
    w2 = np.asarray(w2, dtype=np.float32)
    b2 = np.asarray(b2, dtype=np.float32)

    x_flat = x.reshape(T, D)
    topk_idx = _route(x_flat, noise, router_w, router_b)

    # per-expert token selection (first CAP routed tokens, in token order)
    idx_list = []
    for e in range(E):
        nz = np.flatnonzero((topk_idx == e).any(axis=-1))[:CAP]
        idx_list.append(nz)

    # gather + transpose into [D, TPAD] per expert (dropped/pad slots zero)
    xf_T = np.zeros((D, T + 1), dtype=np.float32)
    xf_T[:, :T] = x_flat.T
    in_maps = []
    for e in range(E):
        xT = np.zeros((D, TPAD), dtype=np.float32)
        nz = idx_list[e]
        xT[:, :len(nz)] = xf_T[:, nz]
        in_maps.append({"xT": xT, "w1": w1[e], "b1": b1[e],
                        "w2": w2[e], "b2": b2[e]})

    nc = _get_nc()
    res = run_bass_kernel_spmd(nc, in_maps, core_ids=list(range(N_CORES)))

    out_flat = np.zeros((T, D), dtype=np.float32)
    for e in range(E):
        nz = idx_list[e]
        out_flat[nz] = res.results[e]["out"][:len(nz)]
    return out_flat.reshape(B, S, D)


# revision 17
# speedup vs baseline: 3.6412x; 1.0370x over previous
"""MoE layer (top-2, E=8, capacity-dropped) on 8 TRN2 NeuronCores.

Strategy (expert-parallel):
  - Router (logits -> softmax -> top-2 -> per-expert capacity selection) runs
    on host via jax CPU, mirroring the reference ops exactly so that top-2
    tie-breaks and capacity cutoffs match the reference bit-for-bit.
    (Router flops are 0.06% of total; the MLPs are the compute.)
  - Token dispatch: per expert e, the first `capacity` routed tokens are
    gathered into a [D, TPAD] transposed activation block (the layout the
    TensorEngine wants for lhsT/rhs streaming).
  - Each of the 8 cores runs one expert's dense MLP:
        out = gelu(x @ w1 + b1) @ w2 + b2        (fp32, ~55 GFLOP/core)
    tiled as: chunk tokens (512) -> layer1 produces H^T [DFF, tc] in SBUF
    (weights streamed), gelu fused on ScalarE with b1 bias, layer2 consumes
    H^T tiles as stationary operands with w2 streamed, bias b2 added on the
    PSUM->SBUF move.
  - Host combine: scatter expert outputs back in expert order (later experts
    overwrite), dropped tokens stay zero.
"""

import numpy as np

B, S, D, DFF, E, TOPK = 8, 2048, 1024, 4096, 8, 2
T = B * S                 # 16384 tokens
CAP = 3277                # ceil(T * 1.6 / 8)
TPAD = 3328               # 26 * 128
NOISE_STD = 0.02
N_CORES = 8
CHUNKS = (512, 512, 512, 512, 512, 512, 256)   # sums to TPAD

_CACHE = {}


def _build_nc(d, dff, tpad, chunks, mm_dt_name="float32", act="Gelu",
              xt_bufs=2, w1_bufs=2, w2_bufs=8, ot_bufs=5,
              ps1_bufs=2, ps2_bufs=1):
    import concourse.mybir as mybir
    import concourse.tile as tile
    from concourse import bacc

    DT = mybir.dt.float32
    MMDT = getattr(mybir.dt, mm_dt_name)
    GELU = getattr(mybir.ActivationFunctionType, act)

    def mm(ap):
        return ap

    kd = d // 128          # k-tiles in D
    nf = dff // 128        # f-tiles in DFF
    nfg = dff // 512       # f-groups of 4
    nd = d // 512          # output n-halves

    nc = bacc.Bacc("TRN2", target_bir_lowering=False, debug=False,
                   num_devices=N_CORES)
    xT_d = nc.dram_tensor("xT", [d, tpad], MMDT, kind="ExternalInput").ap()
    w1_d = nc.dram_tensor("w1", [d, dff], MMDT, kind="ExternalInput").ap()
    b1_d = nc.dram_tensor("b1", [dff], DT, kind="ExternalInput").ap()
    w2_d = nc.dram_tensor("w2", [dff, d], MMDT, kind="ExternalInput").ap()
    b2_d = nc.dram_tensor("b2", [d], DT, kind="ExternalInput").ap()
    out_d = nc.dram_tensor("out", [tpad, d], DT, kind="ExternalOutput").ap()

    with tile.TileContext(nc) as tc:
        with (
            tc.tile_pool(name="consts", bufs=1) as const_pool,
            tc.tile_pool(name="xt", bufs=xt_bufs) as xt_pool,
            tc.tile_pool(name="ht", bufs=1) as ht_pool,
            tc.tile_pool(name="w1p", bufs=w1_bufs) as w1_pool,
            tc.tile_pool(name="w2p", bufs=w2_bufs) as w2_pool,
            tc.tile_pool(name="outp", bufs=ot_bufs) as out_pool,
            tc.tile_pool(name="ps1", bufs=ps1_bufs, space="PSUM") as ps1_pool,
            tc.tile_pool(name="ps2", bufs=ps2_bufs, space="PSUM") as ps2_pool,
        ):
            # biases: b1 as [128, nf] (partition = within-f-tile index),
            # b2 broadcast to all 128 partitions.
            b1_sb = const_pool.tile([128, nf], DT, tag="b1")
            nc.sync.dma_start(b1_sb[:], b1_d.rearrange("(f p) -> p f", p=128))
            b2_row = const_pool.tile([1, d], DT, tag="b2row")
            nc.sync.dma_start(b2_row[:], b2_d.rearrange("(a n) -> a n", a=1))
            b2_sb = const_pool.tile([128, d], DT, tag="b2")
            nc.gpsimd.partition_broadcast(b2_sb[:], b2_row[:])

            c0 = 0
            for tc_sz in chunks:
                ntt = tc_sz // 128
                # ---- load x^T chunk: kd tiles of [128, tc_sz]
                xt = xt_pool.tile([128, kd * tc_sz], MMDT, tag="xt")
                for k in range(kd):
                    nc.sync.dma_start(xt[:, k * tc_sz:(k + 1) * tc_sz],
                                      xT_d[k * 128:(k + 1) * 128, c0:c0 + tc_sz])

                # ---- layer 1: H^T[f-tile, tokens] = gelu(w1_kf.T @ xt_k + b1)
                ht = ht_pool.tile([128, nf * tc_sz], MMDT, tag="ht")
                for fg in range(nfg):
                    w1t = []
                    for k in range(kd):
                        w = w1_pool.tile([128, 512], MMDT, tag=f"w1_{k}")
                        nc.sync.dma_start(
                            w[:], w1_d[k * 128:(k + 1) * 128,
                                       fg * 512:(fg + 1) * 512])
                        w1t.append(w)
                    for fi in range(4):
                        f = fg * 4 + fi
                        ps = ps1_pool.tile([128, tc_sz], DT, tag="ps1")
                        for k in range(kd):
                            nc.tensor.matmul(
                                ps[:],
                                lhsT=mm(w1t[k][:, fi * 128:(fi + 1) * 128]),
                                rhs=mm(xt[:, k * tc_sz:(k + 1) * tc_sz]),
                                start=(k == 0), stop=(k == kd - 1))
                        nc.scalar.activation(
                            ht[:, f * tc_sz:(f + 1) * tc_sz], ps[:], GELU,
                            bias=b1_sb[:, f:f + 1])

                # ---- layer 2: out[tokens, :] = H^T.T @ w2 + b2
                ots = [out_pool.tile([128, d], DT, tag="ot", name="ot")
                       for _ in range(ntt)]
                for n in range(nd):
                    pss = [ps2_pool.tile([128, 512], DT, tag=f"ps2_{t}",
                                         name=f"ps2_{t}")
                           for t in range(ntt)]
                    for f in range(nf):
                        w2t = w2_pool.tile([128, 512], MMDT, tag="w2")
                        nc.sync.dma_start(
                            w2t[:], w2_d[f * 128:(f + 1) * 128,
                                         n * 512:(n + 1) * 512])
                        for t in range(ntt):
                            nc.tensor.matmul(
                                pss[t][:],
                                lhsT=mm(ht[:, f * tc_sz + t * 128:
                                           f * tc_sz + t * 128 + 128]),
                                rhs=mm(w2t[:]),
                                start=(f == 0), stop=(f == nf - 1))
                    for t in range(ntt):
                        nc.vector.tensor_add(
                            ots[t][:, n * 512:(n + 1) * 512], pss[t][:],
                            b2_sb[:, n * 512:(n + 1) * 512])
                for t in range(ntt):
                    nc.sync.dma_start(
                        out_d[c0 + t * 128:c0 + (t + 1) * 128, :], ots[t][:])
                c0 += tc_sz
    nc.compile()
    return nc


MM_DTYPE = "float32r"  # 4x faster PE than float32; ~2e-4 rel absmax error


def _get_nc():
    key = (D, DFF, TPAD, CHUNKS, MM_DTYPE)
    if key not in _CACHE:
        _CACHE[key] = _build_nc(D, DFF, TPAD, CHUNKS, mm_dt_name=MM_DTYPE)
    return _CACHE[key]


def _route(x_flat, noise, router_w, router_b):
    """Mirror of the reference router, on jax CPU (decisions verified to
    match the neuron backend bit-for-bit on this input distribution)."""
    import jax
    import jax.numpy as jnp

    cpu = jax.devices("cpu")[0]
    with jax.default_device(cpu):
        xj = jnp.asarray(x_flat)
        logits = (xj @ jnp.asarray(router_w).T + jnp.asarray(router_b)
                  + jnp.asarray(noise) * NOISE_STD)
        probs = jax.nn.softmax(logits, axis=-1)
        _, topk_idx = jax.lax.top_k(probs, TOPK)
    return np.asarray(topk_idx)


def kernel(x, noise, router_w, router_b, w1, b1, w2, b2):
    from concourse.bass_utils import run_bass_kernel_spmd

    x = np.asarray(x, dtype=np.float32)
    noise = np.asarray(noise, dtype=np.float32)
    router_w = np.asarray(router_w, dtype=np.float32)
    router_b = np.asarray(router_b, dtype=np.float32)
    w1 = np.asarray(w1, dtype=np.float32)
    b1 = np.asarray(b1, dtype=np.float32)
    w2 = np.asarray(w2, dtype=np.float32)
    b2 = np.asarray(b2, dtype=np.float32)

    x_flat = x.reshape(T, D)
    topk_idx = _route(x_flat, noise, router_w, router_b)

    # per-expert token selection (first CAP routed tokens, in token order)
    idx_list = []
    for e in range(E):
        nz = np.flatnonzero((topk_idx == e).any(axis=-1))[:CAP]
        idx_list.append(nz)

    # gather + transpose into [D, TPAD] per expert (dropped/pad slots zero)
    xf_T = np.zeros((D, T + 1), dtype=np.float32)
    xf_T[:, :T] = x_flat.T
    in_maps = []
    for e in range(E):
        xT = np.zeros((D, TPAD), dtype=np.float32)
        nz = idx_list[e]
        xT[:, :len(nz)] = xf_T[:, nz]
        in_maps.append({"xT": xT, "w1": w1[e], "b1": b1[e],
                        "w2": w2[e], "b2": b2[e]})

    nc = _get_nc()
    res = run_bass_kernel_spmd(nc, in_maps, core_ids=list(range(N_CORES)))

    out_flat = np.zeros((T, D), dtype=np.float32)
    for e in range(E):
        nz = idx_list[e]
        out_flat[nz] = res.results[e]["out"][:len(nz)]
    return out_flat.reshape(B, S, D)
